# revision 1
# baseline (speedup 1.0000x reference)
"""Trainium2 Bass kernel for nn_AttentionMoeModel (4-layer attention+MoE transformer).

Sharding across 8 NeuronCores (SPMD, one shared NEFF, per-core data via in_maps):
  - residual stream sequence-sharded (core c owns tokens [128c, 128c+128), token-major,
    fp32), all-gather of normed activations (feature-major, bf16) before each block,
    reduce-scatter (fp32) of block partials after
  - attention head-sharded (core c = head c)
  - dense MLP F-sharded; MoE expert-sharded (core c = expert c, dense over all tokens,
    top-2 combine weight applied as per-partition scale on PSUM->SBUF copy).
    Routing (top-2 selection) is computed in fp32 on each core's resident token slice
    and all-gathered, so expert selection matches the fp32 reference.
  - shared expert F-sharded; lm_head vocab-sharded (per-core output slice)
Big matmuls run bf16 (1 cy/row); small reductions (head-norm column sums, router)
run plain fp32 matmuls. PSUM accumulation is always fp32.
"""
import sys

sys.path.insert(0, "/opt/trn_rl_repo")

from contextlib import ExitStack

import ml_dtypes
import numpy as np

import concourse.bass as bass
import concourse.mybir as mybir
import concourse.tile as tile
from concourse import bacc
from concourse.bass import IndirectOffsetOnAxis
from concourse.bass_utils import run_bass_kernel_spmd
from concourse.masks import make_identity

# model dims (hardcoded per spec)
B, T, D, H, HD, V, L = 1, 1024, 1024, 8, 128, 32000, 4
E, F = 8, 1024
DENSE_N = 2
VE_LAYERS = {0: 0, 3: 1}
WINDOWS = [1024, 512, 1024, 1024]
VE_GATE_CH = 32

NCORE = 8
P = 128
TS = T // NCORE          # 128 tokens per core
ND = D // P              # 8 feature blocks
NT = T // P              # 8 token blocks
VS = V // NCORE          # 4000 vocab per core
CH = 512                 # matmul moving-dim chunk
NCH = T // CH            # 2 chunks
EPS = 1e-6

f32 = mybir.dt.float32
bf16 = mybir.dt.float16  # "bf16" name kept; actually fp16 (8x finer mantissa)
i32 = mybir.dt.int32
AF = mybir.ActivationFunctionType
OP = mybir.AluOpType
AX = mybir.AxisListType
NPBF = np.float16


# ---------------------------------------------------------------- host tables
def _rope_tables():
    inv = 1.0 / (10000.0 ** (np.arange(0, HD, 2, dtype=np.float64) / HD))  # [64]
    fr = np.arange(T, dtype=np.float64)[:, None] * inv[None, :]            # [T, 64]
    cos, sin = np.cos(fr), np.sin(fr)
    cc = np.empty((P, T), np.float32)
    ss = np.empty((P, T), np.float32)
    cc[:64] = cos.T
    cc[64:] = cos.T
    ss[:64] = sin.T
    ss[64:] = -sin.T  # sign baked: rope(t) = t*CC + swap(t)*SS
    return cc, ss


def _block_mask(w, j, ch):
    tk = np.arange(P)[:, None] + P * j
    tq = np.arange(CH)[None, :] + CH * ch
    return ((tk <= tq) & (tq - tk <= w)).astype(np.float32)


def _mask_plan():
    uniq, keys, plan = [], {}, {}
    for w in set(WINDOWS):
        plan[w] = {}
        for j in range(NT):
            for ch in range(NCH):
                m = _block_mask(w, j, ch)
                if not m.any():
                    plan[w][(j, ch)] = "skip"
                elif m.all():
                    plan[w][(j, ch)] = "full"
                else:
                    kb = m.tobytes()
                    if kb not in keys:
                        keys[kb] = len(uniq)
                        uniq.append(m)
                    plan[w][(j, ch)] = keys[kb]
    return np.stack(uniq), plan


MASKS, MASK_PLAN = _mask_plan()
NMASK = MASKS.shape[0]


# ---------------------------------------------------------------- the program
class Builder:
    def __init__(self, nc, tc, ia):
        self.nc = nc
        self.tc = tc
        self.ia = ia
        self.uid = 0

    def name(self, s):
        self.uid += 1
        return f"{s}_{self.uid}"

    def dram(self, s, shape, dtype=f32, shared=False):
        if shared:
            return self.nc.dram_tensor(self.name(s), shape, dtype, addr_space="Shared")
        return self.nc.dram_tensor(self.name(s), shape, dtype)

    # ---- small helpers -----------------------------------------------------
    def rms_tm(self, out_pool, out_tag, x):
        """x [128, D] token-major fp32 -> new fp32 tile rms(x)."""
        nc = self.nc
        scr = self.wk.tile([P, D], f32, name=self.name("rms_scr"), tag="scrD")
        ssq = self.sm.tile([P, 1], f32, name=self.name("ssq"), tag="sm1")
        nc.scalar.activation(scr[:], x[:], AF.Square, accum_out=ssq[:, :1])
        s1 = self.sm.tile([P, 1], f32, name=self.name("rms_s1"), tag="sm1")
        nc.scalar.activation(s1[:], ssq[:], AF.Sqrt, bias=self.eps[:, :1], scale=1.0 / D)
        s2 = self.sm.tile([P, 1], f32, name=self.name("rms_s2"), tag="sm1")
        nc.vector.reciprocal(s2[:], s1[:])
        xn = out_pool.tile([P, D], f32, name=self.name("rms_out"), tag=out_tag)
        nc.scalar.mul(xn[:], x[:], s2[:, :1])
        return xn

    def row_to_tm(self, row):
        """row [1, NT*128] f32 -> [128, NT] token-major via DRAM bounce."""
        nc = self.nc
        db = self.dram("tb", [1, NT * P])
        nc.sync.dma_start(db.ap()[:], row[:])
        out = self.sm.tile([P, NT], f32, name=self.name("tmn"), tag="smn")
        nc.sync.dma_start(out[:], db.ap().rearrange("o (j p) -> (o p) j", p=P)[:])
        return out

    def tm_to_row(self, tm):
        """[128, NT] f32 token-major -> row [1, NT*128] via DRAM bounce."""
        nc = self.nc
        db = self.dram("tb2", [P, NT])
        nc.sync.dma_start(db.ap()[:], tm[:])
        row = self.sm.tile([1, NT * P], f32, name=self.name("rown"), tag="row")
        nc.sync.dma_start(
            row.rearrange("o (j p) -> o j p", p=P)[:],
            db.ap().rearrange("p j -> j p")[:],
        )
        return row

    def bcast(self, row):
        """row [1, T] f32 -> [128, T] partition broadcast."""
        out = self.wk.tile([P, T], f32, name=self.name("bc"), tag="tsw")
        self.nc.gpsimd.partition_broadcast(out[:], row[:])
        return out

    def allgather_fm(self, xn, nm, want_f32T=False, dt=f32):
        """xn [128, D] tm fp32 (my tokens) -> x_fm [128, ND, T] (dt) ('big' pool).
        If want_f32T, also returns my slice transposed in fp32 [128, ND, 128]."""
        nc = self.nc
        xnT = self.wk.tile([P, ND, TS], dt, name=self.name("xnT"), tag="xnT")
        xnT32 = None
        if want_f32T:
            xnT32 = self.wk.tile([P, ND, TS], f32, name=self.name("xnT32"), tag="scrD")
        for db in range(ND):
            pt = self.ps.tile([P, CH], f32, name=self.name("ps_tr"), tag="ps")
            nc.tensor.transpose(pt[:, :P], xn[:, db * P:(db + 1) * P], self.idn[:])
            nc.scalar.copy(xnT[:, db, :], pt[:, :P])
            if want_f32T:
                nc.vector.tensor_copy(xnT32[:, db, :], pt[:, :P])
        agin = self.dram("agin", [D, TS], dt)
        for db in range(ND):
            nc.sync.dma_start(agin.ap()[db * P:(db + 1) * P, :], xnT[:, db, :])
        agout = self.dram("agout", [NCORE * D, TS], dt, shared=True)
        nc.gpsimd.collective_compute(
            "AllGather", OP.bypass, replica_groups=[list(range(NCORE))],
            ins=[agin.ap()[:]], outs=[agout.ap()[:]],
        )
        x_fm = self.big.tile([P, ND, T], dt, name=self.name(nm), tag="big")
        for db in range(ND):
            for r in range(NCORE):
                nc.sync.dma_start(
                    x_fm[:, db, r * TS:(r + 1) * TS],
                    agout.ap()[r * D + db * P: r * D + (db + 1) * P, :],
                )
        return x_fm, xnT32

    def reduce_scatter_add(self, rsin, x):
        nc = self.nc
        rsout = self.dram("rsout", [TS, D])
        nc.gpsimd.collective_compute(
            "ReduceScatter", OP.add, replica_groups=[list(range(NCORE))],
            ins=[rsin.ap()[:]], outs=[rsout.ap()[:]],
        )
        t = self.wk.tile([P, D], f32, name=self.name("rsld"), tag="scrD")
        nc.sync.dma_start(t[:], rsout.ap()[:])
        nc.vector.tensor_add(out=x[:], in0=x[:], in1=t[:])

    # ---- main build --------------------------------------------------------
    def build(self):
        nc, tc, ia = self.nc, self.tc, self.ia
        with ExitStack() as st:
            self.ps = st.enter_context(tc.tile_pool(name="ps", bufs=8, space="PSUM"))
            self.big = st.enter_context(tc.tile_pool(name="big", bufs=2))
            self.sb = st.enter_context(tc.tile_pool(name="sb", bufs=1))
            self.wk = st.enter_context(tc.tile_pool(name="wk", bufs=2))
            self.wblk = st.enter_context(tc.tile_pool(name="wblk", bufs=8))
            self.wrhs = st.enter_context(tc.tile_pool(name="wrhs", bufs=3))
            self.sm = st.enter_context(tc.tile_pool(name="sm", bufs=3))
            self._build_inner()

    def _build_inner(self):
        nc, ia = self.nc, self.ia
        sb, wk, sm = self.sb, self.wk, self.sm

        # constants (persistent)
        self.idn = sb.tile([P, P], f32, name="idn")
        make_identity(nc, self.idn)
        self.eps = sb.tile([P, 1], f32, name="epsc")
        nc.vector.memset(self.eps[:], EPS)
        self.ones = sb.tile([P, 1], f32, name="onesc")
        nc.vector.memset(self.ones[:], 1.0)
        self.ones_bf = sb.tile([P, 1], bf16, name="onesbf")
        nc.vector.memset(self.ones_bf[:], 1.0)
        # constant bias inside attention exp keeps fp16 p in range; cancels in
        # the softmax ratio exactly.
        self.expb = sb.tile([P, 1], f32, name="expbc")
        nc.vector.memset(self.expb[:], -3.0)
        self.cc = sb.tile([P, T], f32, name="ccc")
        nc.sync.dma_start(self.cc[:], ia["cc"][:])
        self.ss = sb.tile([P, T], f32, name="ssc")
        nc.sync.dma_start(self.ss[:], ia["ss"][:])
        self.masks = sb.tile([P, NMASK, CH], f32, name="masksc")
        nc.sync.dma_start(self.masks[:], ia["masks"].rearrange("m p c -> p m c")[:])
        self.lam_r = sb.tile([P, L], f32, name="lamrc")
        nc.sync.dma_start(self.lam_r[:], ia["lam_r"][:])
        self.lam_x = sb.tile([P, L], f32, name="lamxc")
        nc.sync.dma_start(self.lam_x[:], ia["lam_x"][:])
        self.wsel = sb.tile([P, E], f32, name="wselc")
        nc.sync.dma_start(self.wsel[:], ia["wsel"][:])
        self.idx_my = sb.tile([P, 1], i32, name="idxmyc")
        nc.sync.dma_start(self.idx_my[:], ia["idx_my"][:])
        self.idx_all = sb.tile([P, NT], i32, name="idxallc")
        nc.sync.dma_start(self.idx_all[:], ia["idx_all"][:])

        # embedding: x0 = rms(wte[idx_my]); x = x0
        x0g = wk.tile([P, D], f32, name="x0g", tag="scrD")
        nc.gpsimd.indirect_dma_start(
            out=x0g[:], out_offset=None, in_=ia["wte"][:],
            in_offset=IndirectOffsetOnAxis(ap=self.idx_my[:, :1], axis=0),
        )
        x0 = self.rms_tm(sb, "x0slot", x0g)
        x = sb.tile([P, D], f32, name="xres")
        nc.vector.tensor_copy(x[:], x0[:])

        for li in range(L):
            self.layer(li, x, x0)

        # final norm + lm_head (vocab-sharded)
        xf = self.rms_tm(wk, "rmsout", x)
        xf_fm, _ = self.allgather_fm(xf, "xf_fm", dt=bf16)
        off = 0
        while off < VS:
            vw = min(CH, VS - off)
            psums = [self.ps.tile([P, CH], f32, name=self.name("ps_lm"), tag="ps")
                     for _ in range(NT)]
            for db in range(ND):
                wb = self.wrhs.tile([P, CH], bf16, name=self.name("lm_wb"), tag="wrhs")
                nc.sync.dma_start(wb[:, :vw], ia["lmh"][db * P:(db + 1) * P, off:off + vw])
                for tb in range(NT):
                    nc.tensor.matmul(
                        psums[tb][:, :vw],
                        xf_fm[:, db, tb * P:(tb + 1) * P], wb[:, :vw],
                        start=(db == 0), stop=(db == ND - 1),
                    )
            for tb in range(NT):
                ot = wk.tile([P, CH], f32, name=self.name("lm_o"), tag="stg")
                nc.scalar.copy(ot[:, :vw], psums[tb][:, :vw])
                nc.sync.dma_start(ia["out"][tb * P:(tb + 1) * P, off:off + vw], ot[:, :vw])
            off += vw

    # ---- one transformer layer ---------------------------------------------
    def layer(self, li, x, x0):
        nc, ia = self.nc, self.ia
        wk, sm = self.wk, self.sm
        plan = MASK_PLAN[WINDOWS[li]]
        moe_layer = li >= DENSE_N

        # residual mix: x = lam_r[li]*x + lam_x[li]*x0
        t1 = wk.tile([P, D], f32, name=self.name("resmix"), tag="scrD")
        nc.vector.tensor_scalar(out=t1[:], in0=x0[:], scalar1=self.lam_x[:, li:li + 1],
                                scalar2=None, op0=OP.mult)
        nc.vector.scalar_tensor_tensor(out=x[:], in0=x[:], scalar=self.lam_r[:, li:li + 1],
                                       in1=t1[:], op0=OP.mult, op1=OP.add)

        # ---- attention ------------------------------------------------------
        xn = self.rms_tm(wk, "rmsout", x)
        xn_fm, _ = self.allgather_fm(xn, f"xn_fm{li}", dt=f32)

        # per-head rms scale from pre-rope q/k (rotation preserves norms);
        # fp32 column-sum matmuls (tiny ap -> cost irrelevant)
        def head_norm(t_fm, extra):
            sq = wk.tile([P, T], f32, name=self.name("sq"), tag="scrD")
            nc.vector.tensor_tensor(out=sq[:], in0=t_fm[:], in1=t_fm[:], op=OP.mult)
            pr = self.ps.tile([P, CH], f32, name=self.name("ps_hn"), tag="ps")
            for j in range(NT):
                nc.tensor.matmul(pr[:, j:j + 1], sq[:, j * P:(j + 1) * P],
                                 self.ones[:], start=True, stop=True)
            s1 = sm.tile([P, NT], f32, name=self.name("hn1"), tag="smn")
            nc.scalar.activation(s1[:], pr[:, :NT], AF.Sqrt, bias=self.eps[:, :1],
                                 scale=1.0 / HD)
            s2 = sm.tile([P, NT], f32, name=self.name("hn2"), tag="smn")
            nc.vector.reciprocal(s2[:], s1[:])
            if extra != 1.0:
                nc.vector.tensor_scalar(out=s2[:], in0=s2[:], scalar1=extra,
                                        scalar2=None, op0=OP.mult)
            return s2

        def rope_bf(t_fm):
            """fp32 rope on t_fm (in place scratch), bf16 output."""
            tsw = wk.tile([P, T], f32, name=self.name("tsw"), tag="tsw")
            nc.vector.tensor_copy(tsw[0:64, :], t_fm[64:128, :])
            nc.vector.tensor_copy(tsw[64:128, :], t_fm[0:64, :])
            nc.vector.tensor_tensor(out=tsw[:], in0=tsw[:], in1=self.ss[:], op=OP.mult)
            nc.vector.tensor_tensor(out=t_fm[:], in0=t_fm[:], in1=self.cc[:], op=OP.mult)
            a = wk.tile([P, T], f32, name=self.name("rhat"), tag="rhat")
            nc.vector.tensor_add(out=a[:], in0=t_fm[:], in1=tsw[:])
            return a

        def project(nmw):
            """my head's projection xn @ W[:, head] -> fp32 feature-major [128hd, T]"""
            o = wk.tile([P, T], f32, name=self.name(f"prj{nmw}"), tag="qkv")
            for ch in range(NCH):
                pm = self.ps.tile([P, CH], f32, name=self.name("ps_prj"), tag="ps")
                for db in range(ND):
                    blk = self.wblk.tile([P, P], f32, name=self.name("wqkvb"), tag="wblk")
                    nc.sync.dma_start(blk[:], ia[f"w{nmw}"][li, db * P:(db + 1) * P, :])
                    nc.tensor.matmul(
                        pm[:], blk[:], xn_fm[:, db, ch * CH:(ch + 1) * CH],
                        start=(db == 0), stop=(db == ND - 1),
                    )
                nc.scalar.copy(o[:, ch * CH:(ch + 1) * CH], pm[:])
            return o

        # q: project -> head-norm -> scale by rq (pre-rope; commutes) -> rope
        q_fm = project("q")
        rq_tm = head_norm(q_fm, float(HD) ** -0.5)  # fold score scale into rq
        rq_b = self.bcast(self.tm_to_row(rq_tm))
        nc.vector.tensor_tensor(out=q_fm[:], in0=q_fm[:], in1=rq_b[:], op=OP.mult)
        qh = rope_bf(q_fm)
        k_fm = project("k")
        rk_tm = head_norm(k_fm, 1.0)
        kh = rope_bf(k_fm)

        # v: project -> token-major transpose, fused with value-embedding add
        if li in VE_LAYERS:
            vj = VE_LAYERS[li]
            ve_tm = wk.tile([P, NT, P], f32, name=self.name("ve_tm"), tag="vtm")
            for j in range(NT):
                nc.gpsimd.indirect_dma_start(
                    out=ve_tm[:, j, :], out_offset=None, in_=ia[f"ve{vj}"][:],
                    in_offset=IndirectOffsetOnAxis(ap=self.idx_all[:, j:j + 1], axis=0),
                )
            # gate = 2*sigmoid(xn[:, :32] @ vegw)  [1, T] -> token-major [128, NT]
            gate_row = sm.tile([1, T], f32, name=self.name("gate_row"), tag="row")
            for ch in range(NCH):
                pg = self.ps.tile([P, CH], f32, name=self.name("ps_vg"), tag="ps")
                vegw = self.wblk.tile([P, 1], f32, name=self.name("vegwb"), tag="wblk1")
                nc.sync.dma_start(vegw[:], ia["vegw"][vj])
                nc.tensor.matmul(pg[0:1, :], vegw[:],
                                 xn_fm[:, 0, ch * CH:(ch + 1) * CH],
                                 start=True, stop=True)
                nc.scalar.activation(gate_row[:, ch * CH:(ch + 1) * CH], pg[0:1, :],
                                     AF.Sigmoid)
            nc.vector.tensor_scalar(out=gate_row[:], in0=gate_row[:], scalar1=2.0,
                                    scalar2=None, op0=OP.mult)
            gate_tm = self.row_to_tm(gate_row)
        else:
            ve_tm, gate_tm = None, None

        v_fm = project("v")
        v_tm = wk.tile([P, NT, P], f32, name=self.name("v_tm"), tag="vtm")
        for j in range(NT):
            pt = self.ps.tile([P, CH], f32, name=self.name("ps_vt"), tag="ps")
            nc.tensor.transpose(pt[:, :P], v_fm[:, j * P:(j + 1) * P], self.idn[:])
            if ve_tm is None:
                nc.scalar.copy(v_tm[:, j, :], pt[:, :P])
            else:
                # v_tm = gate * ve + v^T   (f32 inputs, bf16 output)
                nc.vector.scalar_tensor_tensor(
                    out=v_tm[:, j, :], in0=ve_tm[:, j, :], scalar=gate_tm[:, j:j + 1],
                    in1=pt[:, :P], op0=OP.mult, op1=OP.add)

        # scores^T -> exp (+mask) -> p [128tk, NT, T] bf16
        p_sb = self.big.tile([P, NT, T], f32, name=self.name("p_sb"), tag="big")
        for j in range(NT):
            for ch in range(NCH):
                kind = plan[(j, ch)]
                if kind == "skip":
                    continue
                pm = self.ps.tile([P, CH], f32, name=self.name("ps_sc"), tag="ps")
                nc.tensor.matmul(pm[:], kh[:, j * P:(j + 1) * P],
                                 qh[:, ch * CH:(ch + 1) * CH], start=True, stop=True)
                dst = p_sb[:, j, ch * CH:(ch + 1) * CH]
                nc.scalar.activation(dst, pm[:], AF.Exp, scale=rk_tm[:, j:j + 1],
                                     bias=self.expb[:, :1])
                if kind != "full":
                    nc.vector.tensor_tensor(out=dst, in0=dst,
                                            in1=self.masks[:, kind, :], op=OP.mult)

        # softmax denominators -> 1/den broadcast row
        den_row = sm.tile([1, T], f32, name=self.name("den_row"), tag="row")
        for ch in range(NCH):
            live = [j for j in range(NT) if plan[(j, ch)] != "skip"]
            pd = self.ps.tile([P, CH], f32, name=self.name("ps_den"), tag="ps")
            for n, j in enumerate(live):
                nc.tensor.matmul(pd[0:1, :], self.ones[:],
                                 p_sb[:, j, ch * CH:(ch + 1) * CH],
                                 start=(n == 0), stop=(n == len(live) - 1))
            nc.scalar.copy(den_row[:, ch * CH:(ch + 1) * CH], pd[0:1, :])
        den_tm = self.row_to_tm(den_row)
        rden_tm = sm.tile([P, NT], f32, name=self.name("rden"), tag="smn")
        nc.vector.reciprocal(rden_tm[:], den_tm[:])
        rden_b = self.bcast(self.tm_to_row(rden_tm))

        # pv -> y [128hd, T] bf16 (normalized)
        y_fm = wk.tile([P, T], f32, name=self.name("y_fm"), tag="rhat")
        for ch in range(NCH):
            live = [j for j in range(NT) if plan[(j, ch)] != "skip"]
            py = self.ps.tile([P, CH], f32, name=self.name("ps_pv"), tag="ps")
            for n, j in enumerate(live):
                nc.tensor.matmul(py[:], v_tm[:, j, :],
                                 p_sb[:, j, ch * CH:(ch + 1) * CH],
                                 start=(n == 0), stop=(n == len(live) - 1))
            nc.vector.tensor_tensor(out=y_fm[:, ch * CH:(ch + 1) * CH], in0=py[:],
                                    in1=rden_b[:, ch * CH:(ch + 1) * CH], op=OP.mult)

        # out-proj partial -> rsin [T, D] fp32 -> RS -> x +=
        wo = wk.tile([P, D], f32, name=self.name("wo_sb"), tag="wo")
        nc.sync.dma_start(wo[:], ia["wo"][li])
        rsin = self.dram("rsin_a", [T, D])
        for tb in range(NT):
            for ch in range(NCH):
                po = self.ps.tile([P, CH], f32, name=self.name("ps_op"), tag="ps")
                nc.tensor.matmul(po[:], y_fm[:, tb * P:(tb + 1) * P],
                                 wo[:, ch * CH:(ch + 1) * CH], start=True, stop=True)
                ot = wk.tile([P, CH], f32, name=self.name("o_stg"), tag="stg")
                nc.scalar.copy(ot[:], po[:])
                nc.sync.dma_start(rsin.ap()[tb * P:(tb + 1) * P, ch * CH:(ch + 1) * CH], ot[:])
        self.reduce_scatter_add(rsin, x)

        # ---- MLP / MoE ------------------------------------------------------
        xm = self.rms_tm(wk, "rmsout", x)
        lowp = li == L - 1  # layer 3 MoE products are post-routing -> fp16
        xm_fm, xmT32 = self.allgather_fm(xm, f"xm_fm{li}", want_f32T=moe_layer,
                                         dt=bf16 if lowp else f32)
        rsin2 = self.dram("rsin_m", [T, D])
        if not moe_layer:
            self.dense_mlp(li, xm_fm, rsin2)
        else:
            self.moe(li - DENSE_N, xm_fm, xmT32, rsin2, bf16 if lowp else f32)
        self.reduce_scatter_add(rsin2, x)

    # ---- dense mlp (F-sharded 512 per core) --------------------------------
    def dense_mlp(self, li, xm_fm, rsin2):
        nc, ia, wk = self.nc, self.ia, self.wk
        NF = 4 * D // NCORE // P  # 4 blocks of my F-shard
        h2 = self.big.tile([P, ND, T], f32, name=self.name("h2"), tag="big")
        for fb in range(NF):
            for ch in range(NCH):
                pm = self.ps.tile([P, CH], f32, name=self.name("ps_fc"), tag="ps")
                for db in range(ND):
                    blk = self.wblk.tile([P, P], f32, name=self.name("fcb"), tag="wblk")
                    nc.sync.dma_start(blk[:], ia["fc_s"][li, db * P:(db + 1) * P,
                                                         fb * P:(fb + 1) * P])
                    nc.tensor.matmul(pm[:], blk[:],
                                     xm_fm[:, db, ch * CH:(ch + 1) * CH],
                                     start=(db == 0), stop=(db == ND - 1))
                ht = wk.tile([P, CH], f32, name=self.name("h_stg"), tag="stg")
                nc.scalar.copy(ht[:], pm[:])
                nc.vector.scalar_tensor_tensor(out=h2[:, fb, ch * CH:(ch + 1) * CH],
                                               in0=ht[:], scalar=0.0, in1=ht[:],
                                               op0=OP.max, op1=OP.mult)
        for ch in range(NCH):
            psums = [self.ps.tile([P, CH], f32, name=self.name("ps_pj"), tag="ps")
                     for _ in range(NT)]
            for fb in range(NF):
                wb = self.wrhs.tile([P, CH], f32, name=self.name("pj_wb"), tag="wrhs")
                nc.sync.dma_start(wb[:], ia["proj_s"][li, fb * P:(fb + 1) * P,
                                                      ch * CH:(ch + 1) * CH])
                for tb in range(NT):
                    nc.tensor.matmul(psums[tb][:], h2[:, fb, tb * P:(tb + 1) * P],
                                     wb[:], start=(fb == 0), stop=(fb == NF - 1))
            for tb in range(NT):
                ot = wk.tile([P, CH], f32, name=self.name("pj_stg"), tag="stg")
                nc.scalar.copy(ot[:], psums[tb][:])
                nc.sync.dma_start(rsin2.ap()[tb * P:(tb + 1) * P,
                                             ch * CH:(ch + 1) * CH], ot[:])

    # ---- MoE (expert-sharded; dense over all tokens) ------------------------
    def moe(self, mi, xm_fm, xmT32, rsin2, mdt):
        nc, ia, wk, sm = self.nc, self.ia, self.wk, self.sm
        # --- routing in fp32 on my resident tokens, then tiny all-gather ---
        rw_sb = sm.tile([P, ND, E], f32, name=self.name("rw_sb"), tag="rw")
        nc.sync.dma_start(rw_sb[:], ia["rw"][mi].rearrange("(n p) e -> p n e", p=P)[:])
        pr = self.ps.tile([P, CH], f32, name=self.name("ps_rt"), tag="ps")
        for db in range(ND):
            nc.tensor.matmul(pr[:, :E], xmT32[:, db, :], rw_sb[:, db, :],
                             start=(db == 0), stop=(db == ND - 1))
        nmax = sm.tile([P, 1], f32, name=self.name("nmax"), tag="sm1")
        nc.vector.tensor_reduce(nmax[:], pr[:, :E], axis=AX.X, op=OP.max, negate=True)
        probs = sm.tile([P, E], f32, name=self.name("probs"), tag="smn")
        se = sm.tile([P, 1], f32, name=self.name("se"), tag="sm1")
        nc.scalar.activation(probs[:], pr[:, :E], AF.Exp, bias=nmax[:, :1],
                             accum_out=se[:, :1])
        rse = sm.tile([P, 1], f32, name=self.name("rse"), tag="sm1")
        nc.vector.reciprocal(rse[:], se[:])
        nc.vector.tensor_scalar(out=probs[:], in0=probs[:], scalar1=rse[:, :1],
                                scalar2=None, op0=OP.mult)
        m8 = sm.tile([P, 8], f32, name=self.name("m8"), tag="smn")
        nc.vector.max(m8[:], probs[:])
        wf_my = sm.tile([P, E], f32, name=self.name("wfmy"), tag="smn")
        nc.vector.tensor_scalar(out=wf_my[:], in0=probs[:], scalar1=m8[:, 1:2],
                                scalar2=None, op0=OP.is_ge)
        nc.vector.tensor_tensor(out=wf_my[:], in0=wf_my[:], in1=probs[:], op=OP.mult)
        wfin = self.dram("wfin", [TS, E])
        nc.sync.dma_start(wfin.ap()[:], wf_my[:])
        wfout = self.dram("wfout", [T, E], shared=True)
        nc.gpsimd.collective_compute(
            "AllGather", OP.bypass, replica_groups=[list(range(NCORE))],
            ins=[wfin.ap()[:]], outs=[wfout.ap()[:]],
        )
        wf_all = sm.tile([P, NT, E], f32, name=self.name("wfall"), tag="wfall")
        nc.sync.dma_start(wf_all[:], wfout.ap().rearrange("(j p) e -> p j e", p=P)[:])
        wcol = sm.tile([P, NT], f32, name=self.name("wcol"), tag="wcol")
        wfsel = sm.tile([P, NT, E], f32, name=self.name("wfsel"), tag="wfall")
        nc.vector.tensor_tensor(out=wfsel[:], in0=wf_all[:],
                                in1=self.wsel[:, None, :].to_broadcast([P, NT, E]),
                                op=OP.mult)
        nc.vector.tensor_reduce(wcol[:], wfsel[:], axis=AX.X, op=OP.add)

        # --- shared expert (F-sharded 128): su = sig_gate * silu(g) * u ------
        g_sb = wk.tile([P, T], f32, name=self.name("g_sb"), tag="sug")
        su = wk.tile([P, T], mdt, name=self.name("su_sb"), tag="sugb")
        gt_row = sm.tile([1, T], f32, name=self.name("gt_row"), tag="row")
        for ch in range(NCH):
            pg = self.ps.tile([P, CH], f32, name=self.name("ps_sg"), tag="ps")
            for db in range(ND):
                gwb = self.wblk.tile([P, 1], mdt, name=self.name("gwb"), tag="wblk1")
                nc.sync.dma_start(gwb[:], ia[f"gatew_{mi}"][db * P:(db + 1) * P, :])
                nc.tensor.matmul(pg[0:1, :], gwb[:],
                                 xm_fm[:, db, ch * CH:(ch + 1) * CH],
                                 start=(db == 0), stop=(db == ND - 1))
            nc.scalar.activation(gt_row[:, ch * CH:(ch + 1) * CH], pg[0:1, :], AF.Sigmoid)
        gt_b = self.bcast(gt_row)
        for gi in range(2):  # 0: g, 1: u
            for ch in range(NCH):
                pm = self.ps.tile([P, CH], f32, name=self.name("ps_gu"), tag="ps")
                for db in range(ND):
                    blk = self.wblk.tile([P, P], mdt, name=self.name("gub"), tag="wblk")
                    nc.sync.dma_start(blk[:], ia[f"gu_s_{mi}"][db * P:(db + 1) * P,
                                                               gi * P:(gi + 1) * P])
                    nc.tensor.matmul(pm[:], blk[:],
                                     xm_fm[:, db, ch * CH:(ch + 1) * CH],
                                     start=(db == 0), stop=(db == ND - 1))
                cs = slice(ch * CH, (ch + 1) * CH)
                if gi == 0:
                    # g_sb = gate * silu(g) = gate * sigmoid(g) * g
                    nc.scalar.activation(g_sb[:, cs], pm[:], AF.Sigmoid)
                    nc.vector.tensor_tensor(out=g_sb[:, cs], in0=g_sb[:, cs], in1=pm[:],
                                            op=OP.mult)
                    nc.vector.tensor_tensor(out=g_sb[:, cs], in0=g_sb[:, cs],
                                            in1=gt_b[:, cs], op=OP.mult)
                else:
                    nc.vector.tensor_tensor(out=su[:, cs], in0=g_sb[:, cs], in1=pm[:],
                                            op=OP.mult)

        # --- routed expert: h = silu(xm @ w1)  bf16 --------------------------
        h = self.big.tile([P, ND, T], mdt, name=self.name("h_moe"), tag="big")
        for fb in range(ND):
            for ch in range(NCH):
                pm = self.ps.tile([P, CH], f32, name=self.name("ps_w1"), tag="ps")
                for db in range(ND):
                    blk = self.wblk.tile([P, P], mdt, name=self.name("w1b"), tag="wblk")
                    nc.sync.dma_start(blk[:], ia[f"w1_{mi}"][db * P:(db + 1) * P,
                                                              fb * P:(fb + 1) * P])
                    nc.tensor.matmul(pm[:], blk[:],
                                     xm_fm[:, db, ch * CH:(ch + 1) * CH],
                                     start=(db == 0), stop=(db == ND - 1))
                cs = slice(ch * CH, (ch + 1) * CH)
                sg = wk.tile([P, CH], f32, name=self.name("sg_stg"), tag="stg")
                nc.scalar.activation(sg[:], pm[:], AF.Sigmoid)
                nc.vector.tensor_tensor(out=h[:, fb, cs], in0=sg[:], in1=pm[:],
                                        op=OP.mult)

        # --- y = wcol * (h @ w2) + su @ down -> rsin2 [T, D] ------------------
        down = wk.tile([P, D], mdt, name=self.name("down_sb"), tag="wo")
        nc.sync.dma_start(down[:], ia[f"down_s_{mi}"][:])
        for ch in range(NCH):
            psums = [self.ps.tile([P, CH], f32, name=self.name("ps_w2"), tag="ps")
                     for _ in range(NT)]
            for fb in range(ND):
                wb = self.wrhs.tile([P, CH], mdt, name=self.name("w2wb"), tag="wrhs")
                nc.sync.dma_start(wb[:], ia[f"w2_{mi}"][fb * P:(fb + 1) * P,
                                                         ch * CH:(ch + 1) * CH])
                for tb in range(NT):
                    nc.tensor.matmul(psums[tb][:], h[:, fb, tb * P:(tb + 1) * P],
                                     wb[:], start=(fb == 0), stop=(fb == ND - 1))
            for tb in range(NT):
                ot = wk.tile([P, CH], f32, name=self.name("moe_stg"), tag="stg")
                nc.scalar.mul(ot[:], psums[tb][:], wcol[:, tb:tb + 1])
                pd = self.ps.tile([P, CH], f32, name=self.name("ps_dn"), tag="ps")
                nc.tensor.matmul(pd[:], su[:, tb * P:(tb + 1) * P],
                                 down[:, ch * CH:(ch + 1) * CH], start=True, stop=True)
                nc.vector.tensor_add(out=ot[:], in0=ot[:], in1=pd[:])
                nc.sync.dma_start(rsin2.ap()[tb * P:(tb + 1) * P,
                                             ch * CH:(ch + 1) * CH], ot[:])


# ---------------------------------------------------------------- build + run
_BUILT = None


def _build():
    global _BUILT
    if _BUILT is not None:
        return _BUILT
    nc = bacc.Bacc("TRN2", target_bir_lowering=False, debug=False, num_devices=NCORE)

    def inp(name, shape, dtype=f32):
        return nc.dram_tensor(name, list(shape), dtype, kind="ExternalInput").ap()

    ia = {
        "idx_my": inp("idx_my", [P, 1], i32),
        "idx_all": inp("idx_all", [P, NT], i32),
        "wte": inp("wte", [V, D]),
        "ve0": inp("ve0", [V, P]),
        "ve1": inp("ve1", [V, P]),
        "vegw": inp("vegw", [2, P, 1]),
        "wq": inp("wq", [L, D, P]),
        "wk": inp("wk", [L, D, P]),
        "wv": inp("wv", [L, D, P]),
        "wo": inp("wo", [L, P, D]),
        "fc_s": inp("fc_s", [DENSE_N, D, 512]),
        "proj_s": inp("proj_s", [DENSE_N, 512, D]),
        "rw": inp("rw", [2, D, E]),
        "wsel": inp("wsel", [P, E]),
        "w1_0": inp("w1_0", [D, F]),
        "w1_1": inp("w1_1", [D, F], bf16),
        "w2_0": inp("w2_0", [F, D]),
        "w2_1": inp("w2_1", [F, D], bf16),
        "gu_s_0": inp("gu_s_0", [D, 2 * P]),
        "gu_s_1": inp("gu_s_1", [D, 2 * P], bf16),
        "down_s_0": inp("down_s_0", [P, D]),
        "down_s_1": inp("down_s_1", [P, D], bf16),
        "gatew_0": inp("gatew_0", [D, 1]),
        "gatew_1": inp("gatew_1", [D, 1], bf16),
        "lmh": inp("lmh", [D, VS], bf16),
        "lam_r": inp("lam_r", [P, L]),
        "lam_x": inp("lam_x", [P, L]),
        "cc": inp("cc", [P, T]),
        "ss": inp("ss", [P, T]),
        "masks": inp("masks", [NMASK, P, CH]),
        "out": nc.dram_tensor("out", [T, VS], f32, kind="ExternalOutput").ap(),
    }
    with tile.TileContext(nc) as tc:
        Builder(nc, tc, ia).build()
    nc.compile()
    _BUILT = nc
    return nc


def _bf(a):
    return np.ascontiguousarray(np.asarray(a)).astype(NPBF)


def make_in_maps(inputs):
    idx = np.asarray(inputs["idx"]).reshape(T).astype(np.int32)
    cc, ss = _rope_tables()
    shared = {
        "idx_all": np.ascontiguousarray(idx.reshape(NT, P).T),
        "wte": np.ascontiguousarray(inputs["wte"], np.float32),
        "rw": np.ascontiguousarray(inputs["router_w"], np.float32),
        "gatew_0": np.ascontiguousarray(np.asarray(inputs["shared_gate_w"])[0], np.float32),
        "gatew_1": _bf(np.asarray(inputs["shared_gate_w"])[1]),
        "lam_r": np.ascontiguousarray(
            np.broadcast_to(np.asarray(inputs["resid_lambdas"], np.float32), (P, L))),
        "lam_x": np.ascontiguousarray(
            np.broadcast_to(np.asarray(inputs["x0_lambdas"], np.float32), (P, L))),
        "cc": cc,
        "ss": ss,
        "masks": MASKS.astype(np.float32),
    }
    in_maps = []
    for c in range(NCORE):
        hs = slice(c * P, (c + 1) * P)
        vegw = np.zeros((2, P, 1), np.float32)
        for j in range(2):
            vegw[j, :VE_GATE_CH, 0] = np.asarray(inputs["ve_gate_w"])[j][:, c]
        gu = np.concatenate(
            [np.asarray(inputs["shared_gu"])[:, :, c * P:(c + 1) * P],
             np.asarray(inputs["shared_gu"])[:, :, F + c * P:F + (c + 1) * P]], axis=2)
        wsel = np.zeros((P, E), np.float32)
        wsel[:, c] = 1.0
        m = dict(shared)
        m.update({
            "idx_my": np.ascontiguousarray(idx[c * P:(c + 1) * P, None]),
            "ve0": np.ascontiguousarray(np.asarray(inputs["ve_tables"])[0][:, hs], np.float32),
            "ve1": np.ascontiguousarray(np.asarray(inputs["ve_tables"])[1][:, hs], np.float32),
            "vegw": vegw,
            "wq": np.ascontiguousarray(np.asarray(inputs["attn_q"])[:, :, hs], np.float32),
            "wk": np.ascontiguousarray(np.asarray(inputs["attn_k"])[:, :, hs], np.float32),
            "wv": np.ascontiguousarray(np.asarray(inputs["attn_v"])[:, :, hs], np.float32),
            "wo": np.ascontiguousarray(np.asarray(inputs["attn_o"])[:, hs, :], np.float32),
            "fc_s": np.ascontiguousarray(
                np.asarray(inputs["mlp_fc"])[:, :, c * 512:(c + 1) * 512], np.float32),
            "proj_s": np.ascontiguousarray(
                np.asarray(inputs["mlp_proj"])[:, c * 512:(c + 1) * 512, :], np.float32),
            "wsel": wsel,
            "w1_0": np.ascontiguousarray(np.asarray(inputs["moe_w1"])[0, c], np.float32),
            "w1_1": _bf(np.asarray(inputs["moe_w1"])[1, c]),
            "w2_0": np.ascontiguousarray(np.asarray(inputs["moe_w2"])[0, c], np.float32),
            "w2_1": _bf(np.asarray(inputs["moe_w2"])[1, c]),
            "gu_s_0": np.ascontiguousarray(gu[0], np.float32),
            "gu_s_1": _bf(gu[1]),
            "down_s_0": np.ascontiguousarray(
                np.asarray(inputs["shared_down"])[0, c * P:(c + 1) * P, :], np.float32),
            "down_s_1": _bf(np.asarray(inputs["shared_down"])[1, c * P:(c + 1) * P, :]),
            "lmh": _bf(np.asarray(inputs["lm_head_w"])[:, c * VS:(c + 1) * VS]),
        })
        in_maps.append(m)
    return in_maps


def kernel(**inputs):
    nc = _build()
    in_maps = make_in_maps(inputs)
    res = run_bass_kernel_spmd(nc, in_maps, list(range(NCORE)))
    outs = [res.results[c]["out"] for c in range(NCORE)]
    return np.concatenate(outs, axis=1).reshape(B, T, V)


if __name__ == "__main__":
    nc = _build()
    n_inst = sum(len(bb.instructions) for bb in nc.main_func.blocks)
    print("build OK; instructions:", n_inst)

